# revision 18
# baseline (speedup 1.0000x reference)
"""GAT layer kernel for Trainium2 (8 NeuronCores, SPMD) — bf16 pipeline, V2.

Math note: the per-destination softmax weights are only used through their
mean over each destination's incoming edges, and a softmax sums to 1, so
attn_w[i] = 1/deg[i] (0 if deg==0) exactly.  The output reduces to:

    out[i] = (agg[i] @ Wv.T + deg[i]*bv) * recip[i],  agg[i] = sum x[row[e]]

Device strategy (dst-node sharded, 49 windows of 128 dst nodes per core):
  - host sorts edges by (group, src half, window) and packs each group's
    edge list CONTIGUOUSLY across window boundaries: 128-slot chunks are
    shared between adjacent windows (the one-hot masks foreign slots
    with col=-1; boundary chunks are accumulated by both windows).  Per
    (group, half) the chunk count is the max across the 8 cores (SPMD
    needs one program), idx-0 padded — ~4% slack vs per-core exact.
    SWDGE descriptor generation (~8ns/row/queue on the 4 Q7 queue
    pairs) is the critical path, so descriptor count ~= packed count.
  - per group of G windows: FOUR dma_gather calls (int16 indices, x
    split into two <32768-row halves) on SWDGE queues 1,2,3,0.  Index
    tables are DMA'd just-in-time per group on the Sync queue, which
    carries nothing else, so prefetch never blocks — this removes the
    21us startup bubble the full-table preload had.
  - per window one wide DVE op builds the one-hots; TensorE accumulates
    aggT[din, dst] += Xg_c^T @ oh_t into PSUM with bf16 matmuls.
  - epilogues (out[dst,:] = (aggT^T @ WvT + deg^T x bv) * recip) are
    DEFERRED one group: their matmuls are issued between groups when
    their inputs are long ready, so TensorE never stalls on the Scalar
    PSUM->SBUF round trip (was ~0.8us/window in the drain).  Scalar
    carries the aggT copies, the recip scale, and the output DMAs.
"""

import os
import numpy as np

P = 128
NCORES = 8
N = 50000
XLO = 25088                   # rows in the low half of x (< 32768 for int16)
XHI = N - XLO
DIN = 128
DOUT = 128
WPC = 49                      # windows per core
NWIN = NCORES * WPC           # 392
NPAD = NWIN * P               # 50176

_last_exec_ns = None
_cache = {}


def _group_sizes():
    # tapered tail: small final groups drain the pipeline quickly
    return [5] * 9 + [4]


def _ensure_ntff_hook():
    import sys
    import types
    if "antenv.axon_hooks" in sys.modules:
        return
    try:
        import antenv
        mod = types.ModuleType("antenv.axon_hooks")
        _h = [None]
        mod.set_axon_ntff_profile_hook = lambda hook: _h.__setitem__(0, hook)
        mod.get_axon_ntff_profile_hook = lambda: _h[0]
        sys.modules["antenv.axon_hooks"] = mod
        antenv.axon_hooks = mod
        from trn_agent_boot.trn_boot import _ntff_profile_via_ctypes
        hook = _ntff_profile_via_ctypes("/opt/axon/libaxon_pjrt.so")
        if hook is not None:
            mod.set_axon_ntff_profile_hook(hook)
    except Exception:
        pass


class Layout:
    """Compile-time (data-dependent, core-common) packing.

    groups: list of dicts with
      nchunks: total xg chunks C_g
      gathers: list of (src_half, cbase_chunks, nchunks) in issue order
      windows: list of (chunklist, colb_off); chunklist = absolute xg
               chunk ids the window accumulates (union across cores)
    tmax: max T_w;  ncid: f32 cols of cidx;  ncolb: colb columns
    idx_f32_off: per-group first f32 column in cidx (+ final sentinel)
    """

    def __init__(self):
        self.groups = []
        self.tmax = 0
        self.ncid = 0
        self.ncolb = 0
        self.idx_f32_off = []

    def key(self):
        parts = [self.tmax, self.ncid, self.ncolb, tuple(self.idx_f32_off)]
        for g in self.groups:
            parts.append((g["nchunks"], tuple(g["gathers"]),
                          tuple((tuple(cl), off) for cl, off in g["windows"])))
        return hash(str(parts))


def _prep(row, col):
    """Host-side packing. Returns (lay, per_core arrays)."""
    row = row.astype(np.int64)
    col = col.astype(np.int64)
    ishi = (row >= XLO).astype(np.int64)

    deg = np.bincount(col, minlength=NPAD).astype(np.float32)
    recip = np.where(deg > 0, 1.0 / np.maximum(deg, 1.0), 0.0).astype(np.float32)

    sizes = _group_sizes()
    NG = len(sizes)
    g0s = np.concatenate([[0], np.cumsum(sizes)[:-1]])

    win = col >> 7
    core = win // WPC
    wloc = win - core * WPC
    dloc = (col & (P - 1)).astype(np.int64)
    wl2g = np.zeros(WPC, np.int64)
    for gi in range(NG):
        wl2g[g0s[gi]:g0s[gi] + sizes[gi]] = gi

    order = np.lexsort((wloc, ishi, wl2g[wloc], core))
    srow, score, shalf, swloc, sd = (row[order], core[order], ishi[order],
                                     wloc[order], dloc[order])
    sg = wl2g[swloc]

    # segment pointers per (core, group, half)
    seg_key = (score * NG + sg) * 2 + shalf
    seg_cnt = np.bincount(seg_key, minlength=NCORES * NG * 2)
    seg_start = np.zeros(NCORES * NG * 2 + 1, np.int64)
    np.cumsum(seg_cnt, out=seg_start[1:])

    lay = Layout()
    # chunk counts per (group, half) = max over cores
    Ch = np.zeros((NG, 2), np.int64)
    for gi in range(NG):
        for h in (0, 1):
            n_max = max(seg_cnt[(c * NG + gi) * 2 + h] for c in range(NCORES))
            Ch[gi, h] = max(1, -(-n_max // P))

    # build groups metadata + per-core data
    idx16_cols = []          # list of per-core [128, ni/16] blocks, per gather
    colb_cols = []           # list of per-core [128] col arrays, per column
    colb_off = 0
    for gi in range(NG):
        C_lo, C_hi = int(Ch[gi, 0]), int(Ch[gi, 1])
        cbase_h = (0, C_lo)
        gathers = []
        for h in (0, 1):
            Chh = (C_lo, C_hi)[h]
            assert Chh >= 2, f"half too small: g{gi} h{h} Chh={Chh}"
            ca = (Chh + 1) // 2
            for sp in ((0, ca), (ca, Chh)):
                gathers.append((h, cbase_h[h] + sp[0], sp[1] - sp[0]))
        # order is (lo A, lo B, hi A, hi B) -> queues 1,2,3,0
        # per-core idx data per gather
        for (h, cb, nchk) in gathers:
            c0 = cb - cbase_h[h]
            blocks = []
            for c in range(NCORES):
                s = seg_start[(c * NG + gi) * 2 + h]
                n = seg_cnt[(c * NG + gi) * 2 + h]
                v = np.zeros(nchk * P, np.int16)
                lo_s, hi_s = c0 * P, c0 * P + nchk * P
                take0, take1 = min(lo_s, n), min(hi_s, n)
                nn = take1 - take0
                if nn > 0:
                    v[:nn] = (srow[s + take0:s + take1]
                              - (XLO if h else 0)).astype(np.int16)
                wrapped = v.reshape(-1, 16).T            # [16, ni/16]
                blocks.append(np.tile(wrapped, (8, 1)))  # [128, ni/16]
            idx16_cols.append(blocks)

        # per-window union spans + col data
        wins = []
        for wl in range(sizes[gi]):
            spans = {0: [None, None], 1: [None, None]}
            percore_rng = np.zeros((NCORES, 2, 2), np.int64)  # [c,h,(s0,s1)]
            for h in (0, 1):
                lo_c, hi_c = None, None
                for c in range(NCORES):
                    s = seg_start[(c * NG + gi) * 2 + h]
                    n = seg_cnt[(c * NG + gi) * 2 + h]
                    wseg = swloc[s:s + n] - g0s[gi]
                    idxs = np.flatnonzero(wseg == wl)
                    if len(idxs) == 0:
                        percore_rng[c, h] = (0, 0)
                        continue
                    s0, s1 = int(idxs[0]), int(idxs[-1]) + 1
                    percore_rng[c, h] = (s0, s1)
                    a, b = s0 // P, -(-s1 // P)
                    lo_c = a if lo_c is None else min(lo_c, a)
                    hi_c = b if hi_c is None else max(hi_c, b)
                spans[h] = [lo_c, hi_c]
            chunklist = []
            ncols = 0
            for h in (0, 1):
                lo_c, hi_c = spans[h]
                if lo_c is None:
                    continue
                for cc in range(lo_c, hi_c):
                    chunklist.append(cbase_h[h] + cc)
                    colv_pc = []
                    for c in range(NCORES):
                        colv = np.full(P, -1.0, np.float32)
                        s = seg_start[(c * NG + gi) * 2 + h]
                        s0, s1 = percore_rng[c, h]
                        if s1 > s0:
                            a = max(s0, cc * P)
                            b = min(s1, (cc + 1) * P)
                            if b > a:
                                colv[a - cc * P:b - cc * P] = \
                                    sd[s + a:s + b].astype(np.float32)
                        colv_pc.append(colv)
                    colb_cols.append(colv_pc)
                    ncols += 1
            assert ncols >= 1, f"empty window g{gi} w{wl}"
            wins.append((chunklist, colb_off))
            colb_off += ncols
            lay.tmax = max(lay.tmax, ncols)
        lay.groups.append({"nchunks": C_lo + C_hi,
                           "gathers": gathers, "windows": wins})

    # cidx assembly: per group blocks, f32 offsets
    off = 0
    per_core_idx = [[] for _ in range(NCORES)]
    bi = 0
    for gi in range(NG):
        lay.idx_f32_off.append(off)
        for _ in lay.groups[gi]["gathers"]:
            blocks = idx16_cols[bi]
            bi += 1
            for c in range(NCORES):
                per_core_idx[c].append(blocks[c])
            off += blocks[0].shape[1] // 2
    lay.idx_f32_off.append(off)
    lay.ncid = off
    lay.ncolb = colb_off

    per_core = []
    for c in range(NCORES):
        idx16 = np.concatenate(per_core_idx[c], axis=1)    # [128, 2*ncid]
        col_map = np.stack([pc[c] for pc in colb_cols], axis=1)  # [128,ncolb]
        rec_map = np.ascontiguousarray(
            recip[c * WPC * P:(c + 1) * WPC * P].reshape(WPC, P).T)
        deg_map = np.ascontiguousarray(
            deg[c * WPC * P:(c + 1) * WPC * P].reshape(1, WPC * P))
        per_core.append((idx16, col_map, rec_map, deg_map))
    return lay, per_core


def _offsets(lay):
    """Column offsets of the packed [P, CW] f32 crest tensor."""
    o = {}
    o["colb"] = 0                                   # bf16, ncolb cols
    o["rec"] = o["colb"] + (lay.ncolb + 1) // 2
    o["wvtb"] = o["rec"] + WPC
    o["iotab"] = o["wvtb"] + DOUT // 2
    o["CW"] = o["iotab"] + P // 2
    # separate 1-partition tensor, loaded on the Scalar queue:
    o["degb"] = 0
    o["bvb"] = o["degb"] + WPC * P // 2
    o["CD"] = o["bvb"] + DOUT // 2
    return o


def _build(lay):
    import concourse.bacc as bacc
    import concourse.mybir as mybir
    from concourse.tile import TileContext

    f32 = mybir.dt.float32
    bf16 = mybir.dt.bfloat16
    i16 = mybir.dt.int16

    o = _offsets(lay)
    CW = o["CW"]

    nc = bacc.Bacc(None, target_bir_lowering=False, num_swdge_queues=4)
    xlo_d = nc.dram_tensor("xlo", [XLO, DIN], bf16, kind="ExternalInput")
    xhi_d = nc.dram_tensor("xhi", [XHI, DIN], bf16, kind="ExternalInput")
    cidx_d = nc.dram_tensor("cidx", [P, lay.ncid], f32, kind="ExternalInput")
    crest_d = nc.dram_tensor("crest", [P, CW], f32, kind="ExternalInput")
    cdeg_d = nc.dram_tensor("cdeg", [1, o["CD"]], f32, kind="ExternalInput")
    out_d = nc.dram_tensor("out", [WPC * P, DOUT], f32, kind="ExternalOutput")

    sizes = _group_sizes()
    g0s = np.concatenate([[0], np.cumsum(sizes)[:-1]])
    QCYC = (1, 2, 3, 0)

    with TileContext(nc) as tc:
        with (
            tc.tile_pool(name="const", bufs=1) as cpool,
            tc.tile_pool(name="idx", bufs=3) as ipool,
            tc.tile_pool(name="xg", bufs=6) as xgpool,
            tc.tile_pool(name="oh", bufs=10) as ohpool,
            tc.tile_pool(name="at", bufs=12) as atpool,
            tc.tile_pool(name="os", bufs=4) as ospool,
            tc.tile_pool(name="ps", bufs=3, space="PSUM") as pspool,
            tc.tile_pool(name="po", bufs=4, space="PSUM") as popool,
        ):
            # group-0 idx tile FIRST on the sync queue: the first gather
            # needs only it; crest loads in the gather's shadow.  A tiny
            # dummy gather (idx from a zeroed tile) warms the Q7 ucode
            # path before the real index data even lands.
            if int(os.environ.get("GAT_DUMMY", "1")):
                dum_sb = cpool.tile([P, 8], f32, tag="dum")
                nc.vector.memset(dum_sb[:], 0.0)
                scratch_sb = cpool.tile([P, P], bf16, tag="scratch")
                nc.gpsimd.dma_gather(
                    out_ap=scratch_sb[:].rearrange("p (c e) -> p c e", e=P),
                    in_ap=xlo_d[:, :],
                    idxs_ap=dum_sb[:].bitcast(i16)[:, 0:8],
                    num_idxs=128, num_idxs_reg=128, elem_size=DIN,
                    single_packet=False, queue_num=1,
                )
            f0, f1 = lay.idx_f32_off[0], lay.idx_f32_off[1]
            idx0_sb = ipool.tile([P, f1 - f0], f32, tag="idx")
            nc.sync.dma_start(out=idx0_sb[:], in_=cidx_d[:, f0:f1])
            crest_sb = cpool.tile([P, CW], f32, tag="crest")
            nc.sync.dma_start(out=crest_sb[:], in_=crest_d[:, :])
            cdeg_sb = cpool.tile([1, o["CD"]], f32, tag="cdeg")
            nc.scalar.dma_start(out=cdeg_sb[:], in_=cdeg_d[:, :])

            colb_sb = crest_sb[:, o["colb"]:o["rec"]].bitcast(bf16)
            rec_sb = crest_sb[:, o["rec"]:o["rec"] + WPC]
            wvtb_sb = crest_sb[:, o["wvtb"]:o["wvtb"] + DOUT // 2].bitcast(bf16)
            iotab_sb = crest_sb[:, o["iotab"]:
                                o["iotab"] + P // 2].bitcast(bf16)
            degb_sb = cdeg_sb[0:1, o["degb"]:
                              o["degb"] + WPC * P // 2].bitcast(bf16)
            bvb_sb = cdeg_sb[0:1, o["bvb"]:o["bvb"] + DOUT // 2].bitcast(bf16)

            def epilogue(wl_abs, aggT_sb):
                out_ps = popool.tile([P, DOUT], f32, tag="outp")
                nc.tensor.matmul(out=out_ps[:], lhsT=aggT_sb[:],
                                 rhs=wvtb_sb[:], start=True, stop=False)
                nc.tensor.matmul(out=out_ps[:],
                                 lhsT=degb_sb[0:1, wl_abs * P:(wl_abs + 1) * P],
                                 rhs=bvb_sb[0:1, :], start=False, stop=True)
                out_sb = ospool.tile([P, DOUT], f32, tag="outs")
                nc.scalar.mul(out=out_sb[:], in_=out_ps[:],
                              mul=rec_sb[:, wl_abs:wl_abs + 1])
                nc.scalar.dma_start(
                    out=out_d[wl_abs * P:(wl_abs + 1) * P, :], in_=out_sb[:])

            pending = []          # deferred (wl_abs, aggT_sb) epilogues
            for gi, g in enumerate(lay.groups):
                C = g["nchunks"]
                if gi == 0:
                    idx_sb = idx0_sb
                else:
                    f0, f1 = lay.idx_f32_off[gi], lay.idx_f32_off[gi + 1]
                    idx_sb = ipool.tile([P, f1 - f0], f32, tag="idx")
                    nc.sync.dma_start(out=idx_sb[:], in_=cidx_d[:, f0:f1])
                idx16_sb = idx_sb[:].bitcast(i16)

                xg = xgpool.tile([P, C * P], bf16, tag="xg")
                xg3 = xg[:].rearrange("p (c e) -> p c e", e=P)
                goff = 0
                for k, (h, cb, nchk) in enumerate(g["gathers"]):
                    ni = nchk * P
                    nc.gpsimd.dma_gather(
                        out_ap=xg3[:, cb:cb + nchk, :],
                        in_ap=(xlo_d if h == 0 else xhi_d)[:, :],
                        idxs_ap=idx16_sb[:, goff:goff + ni // 16],
                        num_idxs=ni,
                        num_idxs_reg=ni,
                        elem_size=DIN,
                        single_packet=bool(int(
                            os.environ.get("GAT_SP", "0"))),
                        queue_num=QCYC[k % 4],
                    )
                    goff += ni // 16

                # flush previous group's epilogues (inputs long ready)
                for wl_abs, at in pending:
                    epilogue(wl_abs, at)
                pending = []

                for wl, (chunklist, coff) in enumerate(g["windows"]):
                    wl_abs = int(g0s[gi]) + wl
                    T_w = len(chunklist)
                    oh = ohpool.tile([P, T_w * P], bf16, tag="oh")
                    nc.vector.tensor_tensor(
                        out=oh[:].rearrange("p (t j) -> p t j", j=P),
                        in0=iotab_sb[:, :P].rearrange(
                            "p (o j) -> p o j", j=P).to_broadcast(
                            [P, T_w, P]),
                        in1=colb_sb[:, coff:coff + T_w].to_broadcast(
                            [P, T_w, P]),
                        op=mybir.AluOpType.is_equal,
                    )
                    agg_ps = pspool.tile([P, P], f32, tag="agg")
                    for t, xc in enumerate(chunklist):
                        nc.tensor.matmul(
                            out=agg_ps[:],
                            lhsT=xg[:, xc * P:(xc + 1) * P],
                            rhs=oh[:, t * P:(t + 1) * P],
                            start=(t == 0),
                            stop=(t == T_w - 1),
                        )
                    aggT_sb = atpool.tile([P, P], bf16, tag="aggT")
                    nc.scalar.copy(out=aggT_sb[:], in_=agg_ps[:])
                    pending.append((wl_abs, aggT_sb))
            for wl_abs, at in pending:
                epilogue(wl_abs, at)
    nc.compile()
    # Rewrite each gather's SWDGE queue as a pure function of its ASSIGNED
    # DMASW sem lane, so every lane is incremented by exactly one queue
    # (the ucode tracks sem ownership per queue).
    lane_q = (1, 2, 3, 0)
    for bb in nc.m.functions[0].blocks:
        for inst in bb.instructions:
            if 'DMAGatherAnt' not in type(inst).__name__:
                continue
            lane = None
            si = inst.sync_info
            if si is not None:
                for u in si.on_update:
                    n = u.ant_name
                    if n and n.startswith('DMASW'):
                        lane = int(n[5:].split('_')[0])
            assert lane is not None, "gather without DMASW sem"
            inst.queue_num = lane_q[lane % 4]
    return nc


def _put_bf16(arr, col_off, data_bf16):
    """Pack a bf16 [rows, n] block into f32 columns of arr at col_off."""
    rows, n = data_bf16.shape
    if n % 2:
        data_bf16 = np.concatenate(
            [data_bf16, np.zeros((rows, 1), data_bf16.dtype)], axis=1)
        n += 1
    tmp = np.zeros((rows, n // 2), np.float32)
    tmp.view(np.uint16).reshape(rows, n)[:] = data_bf16.view(np.uint16)
    arr[:rows, col_off:col_off + n // 2] = tmp


def _pack_const(lay, idx16, col_map, rec_map, deg_map, wvtb, bvb):
    """Returns (cidx, crest, cdeg) arrays for the constant tensors."""
    from ml_dtypes import bfloat16
    o = _offsets(lay)
    assert idx16.shape == (P, lay.ncid * 2), idx16.shape
    cidx = np.ascontiguousarray(idx16).view(np.float32)
    arr = np.zeros((P, o["CW"]), np.float32)
    _put_bf16(arr, o["colb"], col_map.astype(bfloat16))
    arr[:, o["rec"]:o["rec"] + WPC] = rec_map
    _put_bf16(arr, o["wvtb"], wvtb)
    iotab = np.broadcast_to(
        np.arange(P, dtype=np.float32)[None, :], (P, P)).astype(bfloat16)
    _put_bf16(arr, o["iotab"], np.ascontiguousarray(iotab))
    cdg = np.zeros((1, o["CD"]), np.float32)
    _put_bf16(cdg, o["degb"], deg_map.astype(bfloat16))
    _put_bf16(cdg, o["bvb"], bvb)
    return cidx, arr, cdg


def kernel(**inputs):
    global _last_exec_ns
    _ensure_ntff_hook()
    from concourse.bass_utils import run_bass_kernel_spmd
    from ml_dtypes import bfloat16

    x = np.ascontiguousarray(np.asarray(inputs["x"], dtype=np.float32))
    ei = np.asarray(inputs["edge_index"])
    row = np.asarray(ei[0]).astype(np.int64)
    col = np.asarray(ei[1]).astype(np.int64)
    Wv = np.asarray(inputs["Wv"], dtype=np.float32)
    bv = np.asarray(inputs["bv"], dtype=np.float32)

    xb = x.astype(bfloat16)
    wvtb = np.ascontiguousarray(Wv.T).astype(bfloat16)     # [DIN, DOUT]
    bvb = bv.reshape(1, DOUT).astype(bfloat16)

    lay, per_core = _prep(row, col)

    key = lay.key()
    if key not in _cache:
        _cache[key] = _build(lay)
    nc = _cache[key]

    xlo = np.ascontiguousarray(xb[:XLO])
    xhi = np.ascontiguousarray(xb[XLO:])
    in_maps = []
    for c in range(NCORES):
        cidx, crest, cdg = _pack_const(lay, *per_core[c], wvtb, bvb)
        in_maps.append({"xlo": xlo, "xhi": xhi, "cidx": cidx,
                        "crest": crest, "cdeg": cdg})

    trace = bool(os.environ.get("GAT_TRACE"))
    res = run_bass_kernel_spmd(nc, in_maps, list(range(NCORES)), trace=trace)
    _last_exec_ns = res.exec_time_ns
    globals()["_last_res"] = res

    out = np.concatenate([res.results[c]["out"] for c in range(NCORES)], axis=0)
    return np.ascontiguousarray(out[:N])


# revision 19
# speedup vs baseline: 1.0302x; 1.0302x over previous
"""GAT layer kernel for Trainium2 (8 NeuronCores, SPMD) — bf16 pipeline, V2.

Math note: the per-destination softmax weights are only used through their
mean over each destination's incoming edges, and a softmax sums to 1, so
attn_w[i] = 1/deg[i] (0 if deg==0) exactly.  The output reduces to:

    out[i] = (agg[i] @ Wv.T + deg[i]*bv) * recip[i],  agg[i] = sum x[row[e]]

Device strategy (dst-node sharded, 49 windows of 128 dst nodes per core):
  - host sorts edges by (group, src half, window) and packs each group's
    edge list CONTIGUOUSLY across window boundaries: 128-slot chunks are
    shared between adjacent windows (the one-hot masks foreign slots
    with col=-1; boundary chunks are accumulated by both windows).  Per
    (group, half) the chunk count is the max across the 8 cores (SPMD
    needs one program), idx-0 padded — ~4% slack vs per-core exact.
    SWDGE descriptor generation (~8ns/row/queue on the 4 Q7 queue
    pairs) is the critical path, so descriptor count ~= packed count.
  - per group of G windows: FOUR dma_gather calls (int16 indices, x
    split into two <32768-row halves) on SWDGE queues 1,2,3,0.  Index
    tables are DMA'd just-in-time per group on the Sync queue, which
    carries nothing else, so prefetch never blocks — this removes the
    21us startup bubble the full-table preload had.
  - per window one wide DVE op builds the one-hots; TensorE accumulates
    aggT[din, dst] += Xg_c^T @ oh_t into PSUM with bf16 matmuls.
  - epilogues (out[dst,:] = (aggT^T @ WvT + deg^T x bv) * recip) are
    DEFERRED one group: their matmuls are issued between groups when
    their inputs are long ready, so TensorE never stalls on the Scalar
    PSUM->SBUF round trip (was ~0.8us/window in the drain).  Scalar
    carries the aggT copies, the recip scale, and the output DMAs.
"""

import os
import numpy as np

P = 128
NCORES = 8
N = 50000
XLO = 25088                   # rows in the low half of x (< 32768 for int16)
XHI = N - XLO
DIN = 128
DOUT = 128
WPC = 49                      # windows per core
NWIN = NCORES * WPC           # 392
NPAD = NWIN * P               # 50176

_last_exec_ns = None
_cache = {}


def _group_sizes():
    # tapered tail: small final groups drain the pipeline quickly
    return [7] * 6 + [4, 3]


def _ensure_ntff_hook():
    import sys
    import types
    if "antenv.axon_hooks" in sys.modules:
        return
    try:
        import antenv
        mod = types.ModuleType("antenv.axon_hooks")
        _h = [None]
        mod.set_axon_ntff_profile_hook = lambda hook: _h.__setitem__(0, hook)
        mod.get_axon_ntff_profile_hook = lambda: _h[0]
        sys.modules["antenv.axon_hooks"] = mod
        antenv.axon_hooks = mod
        from trn_agent_boot.trn_boot import _ntff_profile_via_ctypes
        hook = _ntff_profile_via_ctypes("/opt/axon/libaxon_pjrt.so")
        if hook is not None:
            mod.set_axon_ntff_profile_hook(hook)
    except Exception:
        pass


class Layout:
    """Compile-time (data-dependent, core-common) packing.

    groups: list of dicts with
      nchunks: total xg chunks C_g
      gathers: list of (src_half, cbase_chunks, nchunks) in issue order
      windows: list of (chunklist, colb_off); chunklist = absolute xg
               chunk ids the window accumulates (union across cores)
    tmax: max T_w;  ncid: f32 cols of cidx;  ncolb: colb columns
    idx_f32_off: per-group first f32 column in cidx (+ final sentinel)
    """

    def __init__(self):
        self.groups = []
        self.tmax = 0
        self.ncid = 0
        self.ncolb = 0
        self.idx_f32_off = []

    def key(self):
        parts = [self.tmax, self.ncid, self.ncolb, tuple(self.idx_f32_off)]
        for g in self.groups:
            parts.append((g["nchunks"], tuple(g["gathers"]),
                          tuple((tuple(cl), off) for cl, off in g["windows"])))
        return hash(str(parts))


def _prep(row, col):
    """Host-side packing. Returns (lay, per_core arrays)."""
    row = row.astype(np.int64)
    col = col.astype(np.int64)
    ishi = (row >= XLO).astype(np.int64)

    deg = np.bincount(col, minlength=NPAD).astype(np.float32)
    recip = np.where(deg > 0, 1.0 / np.maximum(deg, 1.0), 0.0).astype(np.float32)

    sizes = _group_sizes()
    NG = len(sizes)
    g0s = np.concatenate([[0], np.cumsum(sizes)[:-1]])

    win = col >> 7
    core = win // WPC
    wloc = win - core * WPC
    dloc = (col & (P - 1)).astype(np.int64)
    wl2g = np.zeros(WPC, np.int64)
    for gi in range(NG):
        wl2g[g0s[gi]:g0s[gi] + sizes[gi]] = gi

    order = np.lexsort((wloc, ishi, wl2g[wloc], core))
    srow, score, shalf, swloc, sd = (row[order], core[order], ishi[order],
                                     wloc[order], dloc[order])
    sg = wl2g[swloc]

    # segment pointers per (core, group, half)
    seg_key = (score * NG + sg) * 2 + shalf
    seg_cnt = np.bincount(seg_key, minlength=NCORES * NG * 2)
    seg_start = np.zeros(NCORES * NG * 2 + 1, np.int64)
    np.cumsum(seg_cnt, out=seg_start[1:])

    lay = Layout()
    # chunk counts per (group, half) = max over cores
    Ch = np.zeros((NG, 2), np.int64)
    for gi in range(NG):
        for h in (0, 1):
            n_max = max(seg_cnt[(c * NG + gi) * 2 + h] for c in range(NCORES))
            Ch[gi, h] = max(1, -(-n_max // P))

    # build groups metadata + per-core data
    idx16_cols = []          # list of per-core [128, ni/16] blocks, per gather
    colb_cols = []           # list of per-core [128] col arrays, per column
    colb_off = 0
    for gi in range(NG):
        C_lo, C_hi = int(Ch[gi, 0]), int(Ch[gi, 1])
        cbase_h = (0, C_lo)
        gathers = []
        for h in (0, 1):
            Chh = (C_lo, C_hi)[h]
            assert Chh >= 2, f"half too small: g{gi} h{h} Chh={Chh}"
            ca = (Chh + 1) // 2
            for sp in ((0, ca), (ca, Chh)):
                gathers.append((h, cbase_h[h] + sp[0], sp[1] - sp[0]))
        # order is (lo A, lo B, hi A, hi B) -> queues 1,2,3,0
        # per-core idx data per gather
        for (h, cb, nchk) in gathers:
            c0 = cb - cbase_h[h]
            blocks = []
            for c in range(NCORES):
                s = seg_start[(c * NG + gi) * 2 + h]
                n = seg_cnt[(c * NG + gi) * 2 + h]
                v = np.zeros(nchk * P, np.int16)
                lo_s, hi_s = c0 * P, c0 * P + nchk * P
                take0, take1 = min(lo_s, n), min(hi_s, n)
                nn = take1 - take0
                if nn > 0:
                    v[:nn] = (srow[s + take0:s + take1]
                              - (XLO if h else 0)).astype(np.int16)
                wrapped = v.reshape(-1, 16).T            # [16, ni/16]
                blocks.append(np.tile(wrapped, (8, 1)))  # [128, ni/16]
            idx16_cols.append(blocks)

        # per-window union spans + col data
        wins = []
        for wl in range(sizes[gi]):
            spans = {0: [None, None], 1: [None, None]}
            percore_rng = np.zeros((NCORES, 2, 2), np.int64)  # [c,h,(s0,s1)]
            for h in (0, 1):
                lo_c, hi_c = None, None
                for c in range(NCORES):
                    s = seg_start[(c * NG + gi) * 2 + h]
                    n = seg_cnt[(c * NG + gi) * 2 + h]
                    wseg = swloc[s:s + n] - g0s[gi]
                    idxs = np.flatnonzero(wseg == wl)
                    if len(idxs) == 0:
                        percore_rng[c, h] = (0, 0)
                        continue
                    s0, s1 = int(idxs[0]), int(idxs[-1]) + 1
                    percore_rng[c, h] = (s0, s1)
                    a, b = s0 // P, -(-s1 // P)
                    lo_c = a if lo_c is None else min(lo_c, a)
                    hi_c = b if hi_c is None else max(hi_c, b)
                spans[h] = [lo_c, hi_c]
            chunklist = []
            ncols = 0
            for h in (0, 1):
                lo_c, hi_c = spans[h]
                if lo_c is None:
                    continue
                for cc in range(lo_c, hi_c):
                    chunklist.append(cbase_h[h] + cc)
                    colv_pc = []
                    for c in range(NCORES):
                        colv = np.full(P, -1.0, np.float32)
                        s = seg_start[(c * NG + gi) * 2 + h]
                        s0, s1 = percore_rng[c, h]
                        if s1 > s0:
                            a = max(s0, cc * P)
                            b = min(s1, (cc + 1) * P)
                            if b > a:
                                colv[a - cc * P:b - cc * P] = \
                                    sd[s + a:s + b].astype(np.float32)
                        colv_pc.append(colv)
                    colb_cols.append(colv_pc)
                    ncols += 1
            assert ncols >= 1, f"empty window g{gi} w{wl}"
            wins.append((chunklist, colb_off))
            colb_off += ncols
            lay.tmax = max(lay.tmax, ncols)
        lay.groups.append({"nchunks": C_lo + C_hi,
                           "gathers": gathers, "windows": wins})

    # cidx assembly: per group blocks, f32 offsets
    off = 0
    per_core_idx = [[] for _ in range(NCORES)]
    bi = 0
    for gi in range(NG):
        lay.idx_f32_off.append(off)
        for _ in lay.groups[gi]["gathers"]:
            blocks = idx16_cols[bi]
            bi += 1
            for c in range(NCORES):
                per_core_idx[c].append(blocks[c])
            off += blocks[0].shape[1] // 2
    lay.idx_f32_off.append(off)
    lay.ncid = off
    lay.ncolb = colb_off

    per_core = []
    for c in range(NCORES):
        idx16 = np.concatenate(per_core_idx[c], axis=1)    # [128, 2*ncid]
        col_map = np.stack([pc[c] for pc in colb_cols], axis=1)  # [128,ncolb]
        rec_map = np.ascontiguousarray(
            recip[c * WPC * P:(c + 1) * WPC * P].reshape(WPC, P).T)
        deg_map = np.ascontiguousarray(
            deg[c * WPC * P:(c + 1) * WPC * P].reshape(1, WPC * P))
        per_core.append((idx16, col_map, rec_map, deg_map))
    return lay, per_core


def _offsets(lay):
    """Column offsets of the packed [P, CW] f32 crest tensor."""
    o = {}
    o["colb"] = 0                                   # bf16, ncolb cols
    o["rec"] = o["colb"] + (lay.ncolb + 1) // 2
    o["wvtb"] = o["rec"] + WPC
    o["iotab"] = o["wvtb"] + DOUT // 2
    o["CW"] = o["iotab"] + P // 2
    # separate 1-partition tensor, loaded on the Scalar queue:
    o["degb"] = 0
    o["bvb"] = o["degb"] + WPC * P // 2
    o["CD"] = o["bvb"] + DOUT // 2
    return o


def _build(lay):
    import concourse.bacc as bacc
    import concourse.mybir as mybir
    from concourse.tile import TileContext

    f32 = mybir.dt.float32
    bf16 = mybir.dt.bfloat16
    i16 = mybir.dt.int16

    o = _offsets(lay)
    CW = o["CW"]

    nc = bacc.Bacc(None, target_bir_lowering=False, num_swdge_queues=4)
    xlo_d = nc.dram_tensor("xlo", [XLO, DIN], bf16, kind="ExternalInput")
    xhi_d = nc.dram_tensor("xhi", [XHI, DIN], bf16, kind="ExternalInput")
    cidx_d = nc.dram_tensor("cidx", [P, lay.ncid], f32, kind="ExternalInput")
    crest_d = nc.dram_tensor("crest", [P, CW], f32, kind="ExternalInput")
    cdeg_d = nc.dram_tensor("cdeg", [1, o["CD"]], f32, kind="ExternalInput")
    out_d = nc.dram_tensor("out", [WPC * P, DOUT], f32, kind="ExternalOutput")

    sizes = _group_sizes()
    g0s = np.concatenate([[0], np.cumsum(sizes)[:-1]])
    QCYC = (1, 2, 3, 0)

    with TileContext(nc) as tc:
        with (
            tc.tile_pool(name="const", bufs=1) as cpool,
            tc.tile_pool(name="idx", bufs=3) as ipool,
            tc.tile_pool(name="xg", bufs=4) as xgpool,
            tc.tile_pool(name="oh", bufs=10) as ohpool,
            tc.tile_pool(name="at", bufs=16) as atpool,
            tc.tile_pool(name="os", bufs=4) as ospool,
            tc.tile_pool(name="ps", bufs=2, space="PSUM") as pspool,
            tc.tile_pool(name="po", bufs=4, space="PSUM") as popool,
        ):
            # group-0 idx tile FIRST on the sync queue: the first gather
            # needs only it; crest loads in the gather's shadow.  A tiny
            # dummy gather (idx from a zeroed tile) warms the Q7 ucode
            # path before the real index data even lands.
            if int(os.environ.get("GAT_DUMMY", "1")):
                dum_sb = cpool.tile([P, 8], f32, tag="dum")
                nc.vector.memset(dum_sb[:], 0.0)
                scratch_sb = cpool.tile([P, P], bf16, tag="scratch")
                nc.gpsimd.dma_gather(
                    out_ap=scratch_sb[:].rearrange("p (c e) -> p c e", e=P),
                    in_ap=xlo_d[:, :],
                    idxs_ap=dum_sb[:].bitcast(i16)[:, 0:8],
                    num_idxs=128, num_idxs_reg=128, elem_size=DIN,
                    single_packet=False, queue_num=1,
                )
            f0, f1 = lay.idx_f32_off[0], lay.idx_f32_off[1]
            idx0_sb = ipool.tile([P, f1 - f0], f32, tag="idx")
            nc.sync.dma_start(out=idx0_sb[:], in_=cidx_d[:, f0:f1])
            crest_sb = cpool.tile([P, CW], f32, tag="crest")
            nc.sync.dma_start(out=crest_sb[:], in_=crest_d[:, :])
            cdeg_sb = cpool.tile([1, o["CD"]], f32, tag="cdeg")
            nc.scalar.dma_start(out=cdeg_sb[:], in_=cdeg_d[:, :])

            colb_sb = crest_sb[:, o["colb"]:o["rec"]].bitcast(bf16)
            rec_sb = crest_sb[:, o["rec"]:o["rec"] + WPC]
            wvtb_sb = crest_sb[:, o["wvtb"]:o["wvtb"] + DOUT // 2].bitcast(bf16)
            iotab_sb = crest_sb[:, o["iotab"]:
                                o["iotab"] + P // 2].bitcast(bf16)
            degb_sb = cdeg_sb[0:1, o["degb"]:
                              o["degb"] + WPC * P // 2].bitcast(bf16)
            bvb_sb = cdeg_sb[0:1, o["bvb"]:o["bvb"] + DOUT // 2].bitcast(bf16)

            def epilogue(wl_abs, aggT_sb):
                out_ps = popool.tile([P, DOUT], f32, tag="outp")
                nc.tensor.matmul(out=out_ps[:], lhsT=aggT_sb[:],
                                 rhs=wvtb_sb[:], start=True, stop=False)
                nc.tensor.matmul(out=out_ps[:],
                                 lhsT=degb_sb[0:1, wl_abs * P:(wl_abs + 1) * P],
                                 rhs=bvb_sb[0:1, :], start=False, stop=True)
                out_sb = ospool.tile([P, DOUT], f32, tag="outs")
                nc.scalar.mul(out=out_sb[:], in_=out_ps[:],
                              mul=rec_sb[:, wl_abs:wl_abs + 1])
                nc.scalar.dma_start(
                    out=out_d[wl_abs * P:(wl_abs + 1) * P, :], in_=out_sb[:])

            pending = []          # deferred (wl_abs, aggT_sb) epilogues
            for gi, g in enumerate(lay.groups):
                C = g["nchunks"]
                if gi == 0:
                    idx_sb = idx0_sb
                else:
                    f0, f1 = lay.idx_f32_off[gi], lay.idx_f32_off[gi + 1]
                    idx_sb = ipool.tile([P, f1 - f0], f32, tag="idx")
                    nc.sync.dma_start(out=idx_sb[:], in_=cidx_d[:, f0:f1])
                idx16_sb = idx_sb[:].bitcast(i16)

                xg = xgpool.tile([P, C * P], bf16, tag="xg")
                xg3 = xg[:].rearrange("p (c e) -> p c e", e=P)
                goff = 0
                for k, (h, cb, nchk) in enumerate(g["gathers"]):
                    ni = nchk * P
                    nc.gpsimd.dma_gather(
                        out_ap=xg3[:, cb:cb + nchk, :],
                        in_ap=(xlo_d if h == 0 else xhi_d)[:, :],
                        idxs_ap=idx16_sb[:, goff:goff + ni // 16],
                        num_idxs=ni,
                        num_idxs_reg=ni,
                        elem_size=DIN,
                        single_packet=bool(int(
                            os.environ.get("GAT_SP", "0"))),
                        queue_num=QCYC[k % 4],
                    )
                    goff += ni // 16

                # flush previous group's epilogues (inputs long ready)
                for wl_abs, at in pending:
                    epilogue(wl_abs, at)
                pending = []

                for wl, (chunklist, coff) in enumerate(g["windows"]):
                    wl_abs = int(g0s[gi]) + wl
                    T_w = len(chunklist)
                    oh = ohpool.tile([P, T_w * P], bf16, tag="oh")
                    nc.vector.tensor_tensor(
                        out=oh[:].rearrange("p (t j) -> p t j", j=P),
                        in0=iotab_sb[:, :P].rearrange(
                            "p (o j) -> p o j", j=P).to_broadcast(
                            [P, T_w, P]),
                        in1=colb_sb[:, coff:coff + T_w].to_broadcast(
                            [P, T_w, P]),
                        op=mybir.AluOpType.is_equal,
                    )
                    agg_ps = pspool.tile([P, P], f32, tag="agg")
                    for t, xc in enumerate(chunklist):
                        nc.tensor.matmul(
                            out=agg_ps[:],
                            lhsT=xg[:, xc * P:(xc + 1) * P],
                            rhs=oh[:, t * P:(t + 1) * P],
                            start=(t == 0),
                            stop=(t == T_w - 1),
                        )
                    aggT_sb = atpool.tile([P, P], bf16, tag="aggT")
                    nc.scalar.copy(out=aggT_sb[:], in_=agg_ps[:])
                    pending.append((wl_abs, aggT_sb))
            for wl_abs, at in pending:
                epilogue(wl_abs, at)
    nc.compile()
    # Rewrite each gather's SWDGE queue as a pure function of its ASSIGNED
    # DMASW sem lane, so every lane is incremented by exactly one queue
    # (the ucode tracks sem ownership per queue).
    lane_q = (1, 2, 3, 0)
    for bb in nc.m.functions[0].blocks:
        for inst in bb.instructions:
            if 'DMAGatherAnt' not in type(inst).__name__:
                continue
            lane = None
            si = inst.sync_info
            if si is not None:
                for u in si.on_update:
                    n = u.ant_name
                    if n and n.startswith('DMASW'):
                        lane = int(n[5:].split('_')[0])
            assert lane is not None, "gather without DMASW sem"
            inst.queue_num = lane_q[lane % 4]
    return nc


def _put_bf16(arr, col_off, data_bf16):
    """Pack a bf16 [rows, n] block into f32 columns of arr at col_off."""
    rows, n = data_bf16.shape
    if n % 2:
        data_bf16 = np.concatenate(
            [data_bf16, np.zeros((rows, 1), data_bf16.dtype)], axis=1)
        n += 1
    tmp = np.zeros((rows, n // 2), np.float32)
    tmp.view(np.uint16).reshape(rows, n)[:] = data_bf16.view(np.uint16)
    arr[:rows, col_off:col_off + n // 2] = tmp


def _pack_const(lay, idx16, col_map, rec_map, deg_map, wvtb, bvb):
    """Returns (cidx, crest, cdeg) arrays for the constant tensors."""
    from ml_dtypes import bfloat16
    o = _offsets(lay)
    assert idx16.shape == (P, lay.ncid * 2), idx16.shape
    cidx = np.ascontiguousarray(idx16).view(np.float32)
    arr = np.zeros((P, o["CW"]), np.float32)
    _put_bf16(arr, o["colb"], col_map.astype(bfloat16))
    arr[:, o["rec"]:o["rec"] + WPC] = rec_map
    _put_bf16(arr, o["wvtb"], wvtb)
    iotab = np.broadcast_to(
        np.arange(P, dtype=np.float32)[None, :], (P, P)).astype(bfloat16)
    _put_bf16(arr, o["iotab"], np.ascontiguousarray(iotab))
    cdg = np.zeros((1, o["CD"]), np.float32)
    _put_bf16(cdg, o["degb"], deg_map.astype(bfloat16))
    _put_bf16(cdg, o["bvb"], bvb)
    return cidx, arr, cdg


def kernel(**inputs):
    global _last_exec_ns
    _ensure_ntff_hook()
    from concourse.bass_utils import run_bass_kernel_spmd
    from ml_dtypes import bfloat16

    x = np.ascontiguousarray(np.asarray(inputs["x"], dtype=np.float32))
    ei = np.asarray(inputs["edge_index"])
    row = np.asarray(ei[0]).astype(np.int64)
    col = np.asarray(ei[1]).astype(np.int64)
    Wv = np.asarray(inputs["Wv"], dtype=np.float32)
    bv = np.asarray(inputs["bv"], dtype=np.float32)

    xb = x.astype(bfloat16)
    wvtb = np.ascontiguousarray(Wv.T).astype(bfloat16)     # [DIN, DOUT]
    bvb = bv.reshape(1, DOUT).astype(bfloat16)

    lay, per_core = _prep(row, col)

    key = lay.key()
    if key not in _cache:
        _cache[key] = _build(lay)
    nc = _cache[key]

    xlo = np.ascontiguousarray(xb[:XLO])
    xhi = np.ascontiguousarray(xb[XLO:])
    in_maps = []
    for c in range(NCORES):
        cidx, crest, cdg = _pack_const(lay, *per_core[c], wvtb, bvb)
        in_maps.append({"xlo": xlo, "xhi": xhi, "cidx": cidx,
                        "crest": crest, "cdeg": cdg})

    trace = bool(os.environ.get("GAT_TRACE"))
    res = run_bass_kernel_spmd(nc, in_maps, list(range(NCORES)), trace=trace)
    _last_exec_ns = res.exec_time_ns
    globals()["_last_res"] = res

    out = np.concatenate([res.results[c]["out"] for c in range(NCORES)], axis=0)
    return np.ascontiguousarray(out[:N])


# revision 20
# speedup vs baseline: 1.0696x; 1.0383x over previous
"""GAT layer kernel for Trainium2 (8 NeuronCores, SPMD) — bf16 pipeline, V2.

Math note: the per-destination softmax weights are only used through their
mean over each destination's incoming edges, and a softmax sums to 1, so
attn_w[i] = 1/deg[i] (0 if deg==0) exactly.  The output reduces to:

    out[i] = (agg[i] @ Wv.T + deg[i]*bv) * recip[i],  agg[i] = sum x[row[e]]

Device strategy (dst-node sharded, 49 windows of 128 dst nodes per core):
  - host sorts edges by (group, src half, window) and packs each group's
    edge list CONTIGUOUSLY across window boundaries: 128-slot chunks are
    shared between adjacent windows (the one-hot masks foreign slots
    with col=-1; boundary chunks are accumulated by both windows).  Per
    (group, half) the chunk count is the max across the 8 cores (SPMD
    needs one program), idx-0 padded — ~4% slack vs per-core exact.
    SWDGE descriptor generation (~8ns/row/queue on the 4 Q7 queue
    pairs) is the critical path, so descriptor count ~= packed count.
  - per group of G windows: FOUR dma_gather calls (int16 indices, x
    split into two <32768-row halves) on SWDGE queues 1,2,3,0.  Index
    tables are DMA'd just-in-time per group on the Sync queue, which
    carries nothing else, so prefetch never blocks — this removes the
    21us startup bubble the full-table preload had.
  - per window one wide DVE op builds the one-hots; TensorE accumulates
    aggT[din, dst] += Xg_c^T @ oh_t into PSUM with bf16 matmuls.
  - epilogues (out[dst,:] = (aggT^T @ WvT + deg^T x bv) * recip) are
    DEFERRED one group: their matmuls are issued between groups when
    their inputs are long ready, so TensorE never stalls on the Scalar
    PSUM->SBUF round trip (was ~0.8us/window in the drain).  Scalar
    carries the aggT copies, the recip scale, and the output DMAs.
"""

import os
import numpy as np

P = 128
NCORES = 8
N = 50000
XLO = 25088                   # rows in the low half of x (< 32768 for int16)
XHI = N - XLO
DIN = 128
DOUT = 128
WPC = 49                      # windows per core
NWIN = NCORES * WPC           # 392
NPAD = NWIN * P               # 50176

_last_exec_ns = None
_cache = {}


def _group_sizes():
    # tapered tail: small final groups drain the pipeline quickly
    return [7] * 6 + [4, 3]


def _ensure_ntff_hook():
    import sys
    import types
    if "antenv.axon_hooks" in sys.modules:
        return
    try:
        import antenv
        mod = types.ModuleType("antenv.axon_hooks")
        _h = [None]
        mod.set_axon_ntff_profile_hook = lambda hook: _h.__setitem__(0, hook)
        mod.get_axon_ntff_profile_hook = lambda: _h[0]
        sys.modules["antenv.axon_hooks"] = mod
        antenv.axon_hooks = mod
        from trn_agent_boot.trn_boot import _ntff_profile_via_ctypes
        hook = _ntff_profile_via_ctypes("/opt/axon/libaxon_pjrt.so")
        if hook is not None:
            mod.set_axon_ntff_profile_hook(hook)
    except Exception:
        pass


class Layout:
    """Compile-time (data-dependent, core-common) packing.

    groups: list of dicts with
      nchunks: total xg chunks C_g
      gathers: list of (src_half, cbase_chunks, nchunks) in issue order
      windows: list of (chunklist, colb_off); chunklist = absolute xg
               chunk ids the window accumulates (union across cores)
    tmax: max T_w;  ncid: f32 cols of cidx;  ncolb: colb columns
    idx_f32_off: per-group first f32 column in cidx (+ final sentinel)
    """

    def __init__(self):
        self.groups = []
        self.tmax = 0
        self.ncid = 0
        self.ncolb = 0
        self.idx_f32_off = []

    def key(self):
        parts = [self.tmax, self.ncid, self.ncolb, tuple(self.idx_f32_off)]
        for g in self.groups:
            parts.append((g["nchunks"], tuple(g["gathers"]),
                          tuple((tuple(cl), off) for cl, off in g["windows"])))
        return hash(str(parts))


def _prep(row, col):
    """Host-side packing. Returns (lay, per_core arrays)."""
    row = row.astype(np.int64)
    col = col.astype(np.int64)
    ishi = (row >= XLO).astype(np.int64)

    deg = np.bincount(col, minlength=NPAD).astype(np.float32)
    recip = np.where(deg > 0, 1.0 / np.maximum(deg, 1.0), 0.0).astype(np.float32)

    sizes = _group_sizes()
    NG = len(sizes)
    g0s = np.concatenate([[0], np.cumsum(sizes)[:-1]])

    win = col >> 7
    core = win // WPC
    wloc = win - core * WPC
    dloc = (col & (P - 1)).astype(np.int64)
    wl2g = np.zeros(WPC, np.int64)
    for gi in range(NG):
        wl2g[g0s[gi]:g0s[gi] + sizes[gi]] = gi

    order = np.lexsort((wloc, ishi, wl2g[wloc], core))
    srow, score, shalf, swloc, sd = (row[order], core[order], ishi[order],
                                     wloc[order], dloc[order])
    sg = wl2g[swloc]

    # segment pointers per (core, group, half)
    seg_key = (score * NG + sg) * 2 + shalf
    seg_cnt = np.bincount(seg_key, minlength=NCORES * NG * 2)
    seg_start = np.zeros(NCORES * NG * 2 + 1, np.int64)
    np.cumsum(seg_cnt, out=seg_start[1:])

    lay = Layout()
    # chunk counts per (group, half) = max over cores
    Ch = np.zeros((NG, 2), np.int64)
    for gi in range(NG):
        for h in (0, 1):
            n_max = max(seg_cnt[(c * NG + gi) * 2 + h] for c in range(NCORES))
            Ch[gi, h] = max(1, -(-n_max // P))

    # build groups metadata + per-core data
    idx16_cols = []          # list of per-core [128, ni/16] blocks, per gather
    colb_cols = []           # list of per-core [128] col arrays, per column
    colb_off = 0
    for gi in range(NG):
        C_lo, C_hi = int(Ch[gi, 0]), int(Ch[gi, 1])
        cbase_h = (0, C_lo)
        gathers = []
        for h in (0, 1):
            Chh = (C_lo, C_hi)[h]
            assert Chh >= 2, f"half too small: g{gi} h{h} Chh={Chh}"
            ca = (Chh + 1) // 2
            for sp in ((0, ca), (ca, Chh)):
                gathers.append((h, cbase_h[h] + sp[0], sp[1] - sp[0]))
        # order is (lo A, lo B, hi A, hi B) -> queues 1,2,3,0
        # per-core idx data per gather
        for (h, cb, nchk) in gathers:
            c0 = cb - cbase_h[h]
            blocks = []
            for c in range(NCORES):
                s = seg_start[(c * NG + gi) * 2 + h]
                n = seg_cnt[(c * NG + gi) * 2 + h]
                v = np.zeros(nchk * P, np.int16)
                lo_s, hi_s = c0 * P, c0 * P + nchk * P
                take0, take1 = min(lo_s, n), min(hi_s, n)
                nn = take1 - take0
                if nn > 0:
                    v[:nn] = (srow[s + take0:s + take1]
                              - (XLO if h else 0)).astype(np.int16)
                wrapped = v.reshape(-1, 16).T            # [16, ni/16]
                blocks.append(np.tile(wrapped, (8, 1)))  # [128, ni/16]
            idx16_cols.append(blocks)

        # per-window union spans + col data
        wins = []
        for wl in range(sizes[gi]):
            spans = {0: [None, None], 1: [None, None]}
            percore_rng = np.zeros((NCORES, 2, 2), np.int64)  # [c,h,(s0,s1)]
            for h in (0, 1):
                lo_c, hi_c = None, None
                for c in range(NCORES):
                    s = seg_start[(c * NG + gi) * 2 + h]
                    n = seg_cnt[(c * NG + gi) * 2 + h]
                    wseg = swloc[s:s + n] - g0s[gi]
                    idxs = np.flatnonzero(wseg == wl)
                    if len(idxs) == 0:
                        percore_rng[c, h] = (0, 0)
                        continue
                    s0, s1 = int(idxs[0]), int(idxs[-1]) + 1
                    percore_rng[c, h] = (s0, s1)
                    a, b = s0 // P, -(-s1 // P)
                    lo_c = a if lo_c is None else min(lo_c, a)
                    hi_c = b if hi_c is None else max(hi_c, b)
                spans[h] = [lo_c, hi_c]
            chunklist = []
            ncols = 0
            for h in (0, 1):
                lo_c, hi_c = spans[h]
                if lo_c is None:
                    continue
                for cc in range(lo_c, hi_c):
                    chunklist.append(cbase_h[h] + cc)
                    colv_pc = []
                    for c in range(NCORES):
                        colv = np.full(P, -1.0, np.float32)
                        s = seg_start[(c * NG + gi) * 2 + h]
                        s0, s1 = percore_rng[c, h]
                        if s1 > s0:
                            a = max(s0, cc * P)
                            b = min(s1, (cc + 1) * P)
                            if b > a:
                                colv[a - cc * P:b - cc * P] = \
                                    sd[s + a:s + b].astype(np.float32)
                        colv_pc.append(colv)
                    colb_cols.append(colv_pc)
                    ncols += 1
            assert ncols >= 1, f"empty window g{gi} w{wl}"
            wins.append((chunklist, colb_off))
            colb_off += ncols
            lay.tmax = max(lay.tmax, ncols)
        lay.groups.append({"nchunks": C_lo + C_hi,
                           "gathers": gathers, "windows": wins})

    # cidx assembly: per group blocks, f32 offsets
    off = 0
    per_core_idx = [[] for _ in range(NCORES)]
    bi = 0
    for gi in range(NG):
        lay.idx_f32_off.append(off)
        for _ in lay.groups[gi]["gathers"]:
            blocks = idx16_cols[bi]
            bi += 1
            for c in range(NCORES):
                per_core_idx[c].append(blocks[c])
            off += blocks[0].shape[1] // 2
    lay.idx_f32_off.append(off)
    lay.ncid = off
    lay.ncolb = colb_off

    per_core = []
    for c in range(NCORES):
        idx16 = np.concatenate(per_core_idx[c], axis=1)    # [128, 2*ncid]
        col_map = np.stack([pc[c] for pc in colb_cols], axis=1)  # [128,ncolb]
        rec_map = np.ascontiguousarray(
            recip[c * WPC * P:(c + 1) * WPC * P].reshape(WPC, P).T)
        deg_map = np.ascontiguousarray(
            deg[c * WPC * P:(c + 1) * WPC * P].reshape(1, WPC * P))
        per_core.append((idx16, col_map, rec_map, deg_map))
    return lay, per_core


def _offsets(lay):
    """Column offsets of the packed [P, CW] f32 crest tensor."""
    o = {}
    o["colb"] = 0                                   # bf16, ncolb cols
    o["rec"] = o["colb"] + (lay.ncolb + 1) // 2
    o["wvtb"] = o["rec"] + WPC
    o["iotab"] = o["wvtb"] + DOUT // 2
    o["CW"] = o["iotab"] + P // 2
    # separate 1-partition tensor, loaded on the Scalar queue:
    o["degb"] = 0
    o["bvb"] = o["degb"] + WPC * P // 2
    o["CD"] = o["bvb"] + DOUT // 2
    return o


def _build(lay):
    import concourse.bacc as bacc
    import concourse.mybir as mybir
    from concourse.tile import TileContext

    f32 = mybir.dt.float32
    bf16 = mybir.dt.bfloat16
    i16 = mybir.dt.int16

    o = _offsets(lay)
    CW = o["CW"]

    nc = bacc.Bacc(None, target_bir_lowering=False, num_swdge_queues=4)
    xlo_d = nc.dram_tensor("xlo", [XLO, DIN], bf16, kind="ExternalInput")
    xhi_d = nc.dram_tensor("xhi", [XHI, DIN], bf16, kind="ExternalInput")
    cidx_d = nc.dram_tensor("cidx", [P, lay.ncid], f32, kind="ExternalInput")
    crest_d = nc.dram_tensor("crest", [P, CW], f32, kind="ExternalInput")
    cdeg_d = nc.dram_tensor("cdeg", [1, o["CD"]], f32, kind="ExternalInput")
    out_d = nc.dram_tensor("out", [WPC * P, DOUT], f32, kind="ExternalOutput")

    sizes = _group_sizes()
    g0s = np.concatenate([[0], np.cumsum(sizes)[:-1]])
    QCYC = (1, 2, 3, 0)

    with TileContext(nc) as tc:
        with (
            tc.tile_pool(name="const", bufs=1) as cpool,
            tc.tile_pool(name="idx", bufs=3) as ipool,
            tc.tile_pool(name="xg", bufs=4) as xgpool,
            tc.tile_pool(name="oh", bufs=10) as ohpool,
            tc.tile_pool(name="at", bufs=6) as atpool,
            tc.tile_pool(name="os", bufs=4) as ospool,
            tc.tile_pool(name="ps", bufs=2, space="PSUM") as pspool,
            tc.tile_pool(name="po", bufs=4, space="PSUM") as popool,
        ):
            # group-0 idx tile FIRST on the sync queue: the first gather
            # needs only it; crest loads in the gather's shadow.  A tiny
            # dummy gather (idx from a zeroed tile) warms the Q7 ucode
            # path before the real index data even lands.
            if int(os.environ.get("GAT_DUMMY", "1")):
                dum_sb = cpool.tile([P, 8], f32, tag="dum")
                nc.vector.memset(dum_sb[:], 0.0)
                scratch_sb = cpool.tile([P, P], bf16, tag="scratch")
                nc.gpsimd.dma_gather(
                    out_ap=scratch_sb[:].rearrange("p (c e) -> p c e", e=P),
                    in_ap=xlo_d[:, :],
                    idxs_ap=dum_sb[:].bitcast(i16)[:, 0:8],
                    num_idxs=128, num_idxs_reg=128, elem_size=DIN,
                    single_packet=False, queue_num=1,
                )
            f0, f1 = lay.idx_f32_off[0], lay.idx_f32_off[1]
            idx0_sb = ipool.tile([P, f1 - f0], f32, tag="idx")
            nc.sync.dma_start(out=idx0_sb[:], in_=cidx_d[:, f0:f1])
            crest_sb = cpool.tile([P, CW], f32, tag="crest")
            nc.sync.dma_start(out=crest_sb[:], in_=crest_d[:, :])
            cdeg_sb = cpool.tile([1, o["CD"]], f32, tag="cdeg")
            nc.scalar.dma_start(out=cdeg_sb[:], in_=cdeg_d[:, :])

            colb_sb = crest_sb[:, o["colb"]:o["rec"]].bitcast(bf16)
            rec_sb = crest_sb[:, o["rec"]:o["rec"] + WPC]
            wvtb_sb = crest_sb[:, o["wvtb"]:o["wvtb"] + DOUT // 2].bitcast(bf16)
            iotab_sb = crest_sb[:, o["iotab"]:
                                o["iotab"] + P // 2].bitcast(bf16)
            degb_sb = cdeg_sb[0:1, o["degb"]:
                              o["degb"] + WPC * P // 2].bitcast(bf16)
            bvb_sb = cdeg_sb[0:1, o["bvb"]:o["bvb"] + DOUT // 2].bitcast(bf16)

            def epilogue(wl_abs, aggT_sb, q):
                out_ps = popool.tile([P, DOUT], f32, tag="outp")
                nc.tensor.matmul(out=out_ps[:],
                                 lhsT=aggT_sb[:, q * P:(q + 1) * P],
                                 rhs=wvtb_sb[:], start=True, stop=False)
                nc.tensor.matmul(out=out_ps[:],
                                 lhsT=degb_sb[0:1, wl_abs * P:(wl_abs + 1) * P],
                                 rhs=bvb_sb[0:1, :], start=False, stop=True)
                out_sb = ospool.tile([P, DOUT], f32, tag="outs")
                nc.scalar.mul(out=out_sb[:], in_=out_ps[:],
                              mul=rec_sb[:, wl_abs:wl_abs + 1])
                nc.scalar.dma_start(
                    out=out_d[wl_abs * P:(wl_abs + 1) * P, :], in_=out_sb[:])

            pending = []          # deferred (wl_abs, aggT_sb) epilogues
            for gi, g in enumerate(lay.groups):
                C = g["nchunks"]
                if gi == 0:
                    idx_sb = idx0_sb
                else:
                    f0, f1 = lay.idx_f32_off[gi], lay.idx_f32_off[gi + 1]
                    idx_sb = ipool.tile([P, f1 - f0], f32, tag="idx")
                    nc.sync.dma_start(out=idx_sb[:], in_=cidx_d[:, f0:f1])
                idx16_sb = idx_sb[:].bitcast(i16)

                xg = xgpool.tile([P, C * P], bf16, tag="xg")
                xg3 = xg[:].rearrange("p (c e) -> p c e", e=P)
                goff = 0
                for k, (h, cb, nchk) in enumerate(g["gathers"]):
                    ni = nchk * P
                    nc.gpsimd.dma_gather(
                        out_ap=xg3[:, cb:cb + nchk, :],
                        in_ap=(xlo_d if h == 0 else xhi_d)[:, :],
                        idxs_ap=idx16_sb[:, goff:goff + ni // 16],
                        num_idxs=ni,
                        num_idxs_reg=ni,
                        elem_size=DIN,
                        single_packet=bool(int(
                            os.environ.get("GAT_SP", "0"))),
                        queue_num=QCYC[k % 4],
                    )
                    goff += ni // 16

                # flush previous group's epilogues (inputs long ready)
                for args in pending:
                    epilogue(*args)
                pending = []

                # quads: 4 windows accumulate into slices of ONE psum bank
                # (one bank transition per quad instead of per window)
                wins = g["windows"]
                quads = [wins[i:i + 4] for i in range(0, len(wins), 4)]
                wq = 0
                for quad in quads:
                    Q = len(quad)
                    agg_ps = pspool.tile([P, Q * P], f32, tag="agg")
                    aggT_sb = atpool.tile([P, Q * P], bf16, tag="aggT")
                    for q, (chunklist, coff) in enumerate(quad):
                        wl_abs = int(g0s[gi]) + wq
                        wq += 1
                        T_w = len(chunklist)
                        oh = ohpool.tile([P, T_w * P], bf16, tag="oh")
                        nc.vector.tensor_tensor(
                            out=oh[:].rearrange("p (t j) -> p t j", j=P),
                            in0=iotab_sb[:, :P].rearrange(
                                "p (o j) -> p o j", j=P).to_broadcast(
                                [P, T_w, P]),
                            in1=colb_sb[:, coff:coff + T_w].to_broadcast(
                                [P, T_w, P]),
                            op=mybir.AluOpType.is_equal,
                        )
                        for t, xc in enumerate(chunklist):
                            nc.tensor.matmul(
                                out=agg_ps[:, q * P:(q + 1) * P],
                                lhsT=xg[:, xc * P:(xc + 1) * P],
                                rhs=oh[:, t * P:(t + 1) * P],
                                start=(t == 0),
                                stop=(t == T_w - 1),
                            )
                        pending.append((wl_abs, aggT_sb, q))
                    nc.scalar.copy(out=aggT_sb[:], in_=agg_ps[:])
            for args in pending:
                epilogue(*args)
    nc.compile()
    # Rewrite each gather's SWDGE queue as a pure function of its ASSIGNED
    # DMASW sem lane, so every lane is incremented by exactly one queue
    # (the ucode tracks sem ownership per queue).
    lane_q = (1, 2, 3, 0)
    for bb in nc.m.functions[0].blocks:
        for inst in bb.instructions:
            if 'DMAGatherAnt' not in type(inst).__name__:
                continue
            lane = None
            si = inst.sync_info
            if si is not None:
                for u in si.on_update:
                    n = u.ant_name
                    if n and n.startswith('DMASW'):
                        lane = int(n[5:].split('_')[0])
            assert lane is not None, "gather without DMASW sem"
            inst.queue_num = lane_q[lane % 4]
    return nc


def _put_bf16(arr, col_off, data_bf16):
    """Pack a bf16 [rows, n] block into f32 columns of arr at col_off."""
    rows, n = data_bf16.shape
    if n % 2:
        data_bf16 = np.concatenate(
            [data_bf16, np.zeros((rows, 1), data_bf16.dtype)], axis=1)
        n += 1
    tmp = np.zeros((rows, n // 2), np.float32)
    tmp.view(np.uint16).reshape(rows, n)[:] = data_bf16.view(np.uint16)
    arr[:rows, col_off:col_off + n // 2] = tmp


def _pack_const(lay, idx16, col_map, rec_map, deg_map, wvtb, bvb):
    """Returns (cidx, crest, cdeg) arrays for the constant tensors."""
    from ml_dtypes import bfloat16
    o = _offsets(lay)
    assert idx16.shape == (P, lay.ncid * 2), idx16.shape
    cidx = np.ascontiguousarray(idx16).view(np.float32)
    arr = np.zeros((P, o["CW"]), np.float32)
    _put_bf16(arr, o["colb"], col_map.astype(bfloat16))
    arr[:, o["rec"]:o["rec"] + WPC] = rec_map
    _put_bf16(arr, o["wvtb"], wvtb)
    iotab = np.broadcast_to(
        np.arange(P, dtype=np.float32)[None, :], (P, P)).astype(bfloat16)
    _put_bf16(arr, o["iotab"], np.ascontiguousarray(iotab))
    cdg = np.zeros((1, o["CD"]), np.float32)
    _put_bf16(cdg, o["degb"], deg_map.astype(bfloat16))
    _put_bf16(cdg, o["bvb"], bvb)
    return cidx, arr, cdg


def kernel(**inputs):
    global _last_exec_ns
    _ensure_ntff_hook()
    from concourse.bass_utils import run_bass_kernel_spmd
    from ml_dtypes import bfloat16

    x = np.ascontiguousarray(np.asarray(inputs["x"], dtype=np.float32))
    ei = np.asarray(inputs["edge_index"])
    row = np.asarray(ei[0]).astype(np.int64)
    col = np.asarray(ei[1]).astype(np.int64)
    Wv = np.asarray(inputs["Wv"], dtype=np.float32)
    bv = np.asarray(inputs["bv"], dtype=np.float32)

    xb = x.astype(bfloat16)
    wvtb = np.ascontiguousarray(Wv.T).astype(bfloat16)     # [DIN, DOUT]
    bvb = bv.reshape(1, DOUT).astype(bfloat16)

    lay, per_core = _prep(row, col)

    key = lay.key()
    if key not in _cache:
        _cache[key] = _build(lay)
    nc = _cache[key]

    xlo = np.ascontiguousarray(xb[:XLO])
    xhi = np.ascontiguousarray(xb[XLO:])
    in_maps = []
    for c in range(NCORES):
        cidx, crest, cdg = _pack_const(lay, *per_core[c], wvtb, bvb)
        in_maps.append({"xlo": xlo, "xhi": xhi, "cidx": cidx,
                        "crest": crest, "cdeg": cdg})

    trace = bool(os.environ.get("GAT_TRACE"))
    res = run_bass_kernel_spmd(nc, in_maps, list(range(NCORES)), trace=trace)
    _last_exec_ns = res.exec_time_ns
    globals()["_last_res"] = res

    out = np.concatenate([res.results[c]["out"] for c in range(NCORES)], axis=0)
    return np.ascontiguousarray(out[:N])


# revision 21
# speedup vs baseline: 1.0965x; 1.0251x over previous
"""GAT layer kernel for Trainium2 (8 NeuronCores, SPMD) — bf16 pipeline, V2.

Math note: the per-destination softmax weights are only used through their
mean over each destination's incoming edges, and a softmax sums to 1, so
attn_w[i] = 1/deg[i] (0 if deg==0) exactly.  The output reduces to:

    out[i] = (agg[i] @ Wv.T + deg[i]*bv) * recip[i],  agg[i] = sum x[row[e]]

Device strategy (dst-node sharded, 49 windows of 128 dst nodes per core),
~255us vs the 293us per-window-padded predecessor:
  - host sorts edges by (group, src half, window) and packs each group's
    edge list CONTIGUOUSLY across window boundaries: 128-slot chunks are
    shared between adjacent windows (the one-hot masks foreign slots
    with col=-1; boundary chunks are accumulated by both windows).  Per
    (group, half) the chunk count is the max across the 8 cores (SPMD
    needs one program), idx-0 padded — ~4% slack vs per-core exact,
    ~8% fewer descriptors than per-window chunk padding.  SWDGE
    descriptor generation (~7.8ns/row/queue + ~1us/instr on the 4 Q7
    queue pairs, hard ucode limit MAX_SWDGE_QUEUES=4) is the critical
    path: ~103.2k descriptors/core -> ~214us of gen.
  - per group of 7 windows: FOUR dma_gather calls (int16 indices, x
    split into two <32768-row halves) on SWDGE queues 1,2,3,0.  Index
    tables are DMA'd just-in-time per group on the Sync queue, which
    carries nothing else, so prefetch never blocks.  A 128-idx dummy
    gather issues first to absorb part of the ~11us first-extended-
    instruction ucode warm-up.  single_packet=True corrupts the device
    for this shape — keep False.
  - per window one wide DVE op builds the one-hots (single-column iota
    + col broadcast APs keep the const tensor tiny); TensorE
    accumulates aggT[din, dst] += Xg_c^T @ oh_t with bf16 matmuls.
    QUADS of up to 4 windows accumulate into column slices of ONE PSUM
    bank: one bank transition per quad instead of per window removes a
    ~0.8us/window PE bubble in the drain (bank switches are expensive,
    ~12us total win).  One Scalar PSUM->SBUF copy per quad.
  - epilogues (out[dst,:] = (aggT^T @ WvT + deg^T x bv) * recip) are
    DEFERRED one group: their matmuls are issued between groups when
    their inputs are long ready, so TensorE never stalls on the Scalar
    round trip.  Scalar carries the quad copies, the recip scale, and
    the output DMAs (keeping Sync free for index prefetch).
  - deep oh buffering (bufs=10) pre-builds one-hots during the gather
    phase, so the post-gather drain is TensorE-paced (~2.4us/window
    over the last ~7 windows) rather than DVE-paced.
"""

import os
import numpy as np

P = 128
NCORES = 8
N = 50000
XLO = 25088                   # rows in the low half of x (< 32768 for int16)
XHI = N - XLO
DIN = 128
DOUT = 128
WPC = 49                      # windows per core
NWIN = NCORES * WPC           # 392
NPAD = NWIN * P               # 50176

_last_exec_ns = None
_cache = {}


def _group_sizes():
    # tapered tail: small final groups drain the pipeline quickly
    return [7] * 6 + [4, 3]


def _ensure_ntff_hook():
    import sys
    import types
    if "antenv.axon_hooks" in sys.modules:
        return
    try:
        import antenv
        mod = types.ModuleType("antenv.axon_hooks")
        _h = [None]
        mod.set_axon_ntff_profile_hook = lambda hook: _h.__setitem__(0, hook)
        mod.get_axon_ntff_profile_hook = lambda: _h[0]
        sys.modules["antenv.axon_hooks"] = mod
        antenv.axon_hooks = mod
        from trn_agent_boot.trn_boot import _ntff_profile_via_ctypes
        hook = _ntff_profile_via_ctypes("/opt/axon/libaxon_pjrt.so")
        if hook is not None:
            mod.set_axon_ntff_profile_hook(hook)
    except Exception:
        pass


class Layout:
    """Compile-time (data-dependent, core-common) packing.

    groups: list of dicts with
      nchunks: total xg chunks C_g
      gathers: list of (src_half, cbase_chunks, nchunks) in issue order
      windows: list of (chunklist, colb_off); chunklist = absolute xg
               chunk ids the window accumulates (union across cores)
    tmax: max T_w;  ncid: f32 cols of cidx;  ncolb: colb columns
    idx_f32_off: per-group first f32 column in cidx (+ final sentinel)
    """

    def __init__(self):
        self.groups = []
        self.tmax = 0
        self.ncid = 0
        self.ncolb = 0
        self.idx_f32_off = []

    def key(self):
        parts = [self.tmax, self.ncid, self.ncolb, tuple(self.idx_f32_off)]
        for g in self.groups:
            parts.append((g["nchunks"], tuple(g["gathers"]),
                          tuple((tuple(cl), off) for cl, off in g["windows"])))
        return hash(str(parts))


def _prep(row, col):
    """Host-side packing. Returns (lay, per_core arrays)."""
    row = row.astype(np.int64)
    col = col.astype(np.int64)
    ishi = (row >= XLO).astype(np.int64)

    deg = np.bincount(col, minlength=NPAD).astype(np.float32)
    recip = np.where(deg > 0, 1.0 / np.maximum(deg, 1.0), 0.0).astype(np.float32)

    sizes = _group_sizes()
    NG = len(sizes)
    g0s = np.concatenate([[0], np.cumsum(sizes)[:-1]])

    win = col >> 7
    core = win // WPC
    wloc = win - core * WPC
    dloc = (col & (P - 1)).astype(np.int64)
    wl2g = np.zeros(WPC, np.int64)
    for gi in range(NG):
        wl2g[g0s[gi]:g0s[gi] + sizes[gi]] = gi

    order = np.lexsort((wloc, ishi, wl2g[wloc], core))
    srow, score, shalf, swloc, sd = (row[order], core[order], ishi[order],
                                     wloc[order], dloc[order])
    sg = wl2g[swloc]

    # segment pointers per (core, group, half)
    seg_key = (score * NG + sg) * 2 + shalf
    seg_cnt = np.bincount(seg_key, minlength=NCORES * NG * 2)
    seg_start = np.zeros(NCORES * NG * 2 + 1, np.int64)
    np.cumsum(seg_cnt, out=seg_start[1:])

    lay = Layout()
    # chunk counts per (group, half) = max over cores
    Ch = np.zeros((NG, 2), np.int64)
    for gi in range(NG):
        for h in (0, 1):
            n_max = max(seg_cnt[(c * NG + gi) * 2 + h] for c in range(NCORES))
            Ch[gi, h] = max(1, -(-n_max // P))

    # build groups metadata + per-core data
    idx16_cols = []          # list of per-core [128, ni/16] blocks, per gather
    colb_cols = []           # list of per-core [128] col arrays, per column
    colb_off = 0
    for gi in range(NG):
        C_lo, C_hi = int(Ch[gi, 0]), int(Ch[gi, 1])
        cbase_h = (0, C_lo)
        gathers = []
        for h in (0, 1):
            Chh = (C_lo, C_hi)[h]
            assert Chh >= 2, f"half too small: g{gi} h{h} Chh={Chh}"
            ca = (Chh + 1) // 2
            for sp in ((0, ca), (ca, Chh)):
                gathers.append((h, cbase_h[h] + sp[0], sp[1] - sp[0]))
        # order is (lo A, lo B, hi A, hi B) -> queues 1,2,3,0
        # per-core idx data per gather
        for (h, cb, nchk) in gathers:
            c0 = cb - cbase_h[h]
            blocks = []
            for c in range(NCORES):
                s = seg_start[(c * NG + gi) * 2 + h]
                n = seg_cnt[(c * NG + gi) * 2 + h]
                v = np.zeros(nchk * P, np.int16)
                lo_s, hi_s = c0 * P, c0 * P + nchk * P
                take0, take1 = min(lo_s, n), min(hi_s, n)
                nn = take1 - take0
                if nn > 0:
                    v[:nn] = (srow[s + take0:s + take1]
                              - (XLO if h else 0)).astype(np.int16)
                wrapped = v.reshape(-1, 16).T            # [16, ni/16]
                blocks.append(np.tile(wrapped, (8, 1)))  # [128, ni/16]
            idx16_cols.append(blocks)

        # per-window union spans + col data
        wins = []
        for wl in range(sizes[gi]):
            spans = {0: [None, None], 1: [None, None]}
            percore_rng = np.zeros((NCORES, 2, 2), np.int64)  # [c,h,(s0,s1)]
            for h in (0, 1):
                lo_c, hi_c = None, None
                for c in range(NCORES):
                    s = seg_start[(c * NG + gi) * 2 + h]
                    n = seg_cnt[(c * NG + gi) * 2 + h]
                    wseg = swloc[s:s + n] - g0s[gi]
                    idxs = np.flatnonzero(wseg == wl)
                    if len(idxs) == 0:
                        percore_rng[c, h] = (0, 0)
                        continue
                    s0, s1 = int(idxs[0]), int(idxs[-1]) + 1
                    percore_rng[c, h] = (s0, s1)
                    a, b = s0 // P, -(-s1 // P)
                    lo_c = a if lo_c is None else min(lo_c, a)
                    hi_c = b if hi_c is None else max(hi_c, b)
                spans[h] = [lo_c, hi_c]
            chunklist = []
            ncols = 0
            for h in (0, 1):
                lo_c, hi_c = spans[h]
                if lo_c is None:
                    continue
                for cc in range(lo_c, hi_c):
                    chunklist.append(cbase_h[h] + cc)
                    colv_pc = []
                    for c in range(NCORES):
                        colv = np.full(P, -1.0, np.float32)
                        s = seg_start[(c * NG + gi) * 2 + h]
                        s0, s1 = percore_rng[c, h]
                        if s1 > s0:
                            a = max(s0, cc * P)
                            b = min(s1, (cc + 1) * P)
                            if b > a:
                                colv[a - cc * P:b - cc * P] = \
                                    sd[s + a:s + b].astype(np.float32)
                        colv_pc.append(colv)
                    colb_cols.append(colv_pc)
                    ncols += 1
            assert ncols >= 1, f"empty window g{gi} w{wl}"
            wins.append((chunklist, colb_off))
            colb_off += ncols
            lay.tmax = max(lay.tmax, ncols)
        lay.groups.append({"nchunks": C_lo + C_hi,
                           "gathers": gathers, "windows": wins})

    # cidx assembly: per group blocks, f32 offsets
    off = 0
    per_core_idx = [[] for _ in range(NCORES)]
    bi = 0
    for gi in range(NG):
        lay.idx_f32_off.append(off)
        for _ in lay.groups[gi]["gathers"]:
            blocks = idx16_cols[bi]
            bi += 1
            for c in range(NCORES):
                per_core_idx[c].append(blocks[c])
            off += blocks[0].shape[1] // 2
    lay.idx_f32_off.append(off)
    lay.ncid = off
    lay.ncolb = colb_off

    per_core = []
    for c in range(NCORES):
        idx16 = np.concatenate(per_core_idx[c], axis=1)    # [128, 2*ncid]
        col_map = np.stack([pc[c] for pc in colb_cols], axis=1)  # [128,ncolb]
        rec_map = np.ascontiguousarray(
            recip[c * WPC * P:(c + 1) * WPC * P].reshape(WPC, P).T)
        deg_map = np.ascontiguousarray(
            deg[c * WPC * P:(c + 1) * WPC * P].reshape(1, WPC * P))
        per_core.append((idx16, col_map, rec_map, deg_map))
    return lay, per_core


def _offsets(lay):
    """Column offsets of the packed [P, CW] f32 crest tensor."""
    o = {}
    o["colb"] = 0                                   # bf16, ncolb cols
    o["rec"] = o["colb"] + (lay.ncolb + 1) // 2
    o["wvtb"] = o["rec"] + WPC
    o["iotab"] = o["wvtb"] + DOUT // 2
    o["CW"] = o["iotab"] + P // 2
    # separate 1-partition tensor, loaded on the Scalar queue:
    o["degb"] = 0
    o["bvb"] = o["degb"] + WPC * P // 2
    o["CD"] = o["bvb"] + DOUT // 2
    return o


def _build(lay):
    import concourse.bacc as bacc
    import concourse.mybir as mybir
    from concourse.tile import TileContext

    f32 = mybir.dt.float32
    bf16 = mybir.dt.bfloat16
    i16 = mybir.dt.int16

    o = _offsets(lay)
    CW = o["CW"]

    nc = bacc.Bacc(None, target_bir_lowering=False, num_swdge_queues=4)
    xlo_d = nc.dram_tensor("xlo", [XLO, DIN], bf16, kind="ExternalInput")
    xhi_d = nc.dram_tensor("xhi", [XHI, DIN], bf16, kind="ExternalInput")
    cidx_d = nc.dram_tensor("cidx", [P, lay.ncid], f32, kind="ExternalInput")
    crest_d = nc.dram_tensor("crest", [P, CW], f32, kind="ExternalInput")
    cdeg_d = nc.dram_tensor("cdeg", [1, o["CD"]], f32, kind="ExternalInput")
    out_d = nc.dram_tensor("out", [WPC * P, DOUT], f32, kind="ExternalOutput")

    sizes = _group_sizes()
    g0s = np.concatenate([[0], np.cumsum(sizes)[:-1]])
    QCYC = (1, 2, 3, 0)

    with TileContext(nc) as tc:
        with (
            tc.tile_pool(name="const", bufs=1) as cpool,
            tc.tile_pool(name="idx", bufs=3) as ipool,
            tc.tile_pool(name="xg", bufs=4) as xgpool,
            tc.tile_pool(name="oh", bufs=10) as ohpool,
            tc.tile_pool(name="at", bufs=6) as atpool,
            tc.tile_pool(name="os", bufs=4) as ospool,
            tc.tile_pool(name="ps", bufs=2, space="PSUM") as pspool,
            tc.tile_pool(name="po", bufs=4, space="PSUM") as popool,
        ):
            # group-0 idx tile FIRST on the sync queue: the first gather
            # needs only it; crest loads in the gather's shadow.  A tiny
            # dummy gather (idx from a zeroed tile) warms the Q7 ucode
            # path before the real index data even lands.
            if int(os.environ.get("GAT_DUMMY", "1")):
                dum_sb = cpool.tile([P, 8], f32, tag="dum")
                nc.vector.memset(dum_sb[:], 0.0)
                scratch_sb = cpool.tile([P, P], bf16, tag="scratch")
                nc.gpsimd.dma_gather(
                    out_ap=scratch_sb[:].rearrange("p (c e) -> p c e", e=P),
                    in_ap=xlo_d[:, :],
                    idxs_ap=dum_sb[:].bitcast(i16)[:, 0:8],
                    num_idxs=128, num_idxs_reg=128, elem_size=DIN,
                    single_packet=False, queue_num=1,
                )
            f0, f1 = lay.idx_f32_off[0], lay.idx_f32_off[1]
            idx0_sb = ipool.tile([P, f1 - f0], f32, tag="idx")
            nc.sync.dma_start(out=idx0_sb[:], in_=cidx_d[:, f0:f1])
            crest_sb = cpool.tile([P, CW], f32, tag="crest")
            nc.sync.dma_start(out=crest_sb[:], in_=crest_d[:, :])
            cdeg_sb = cpool.tile([1, o["CD"]], f32, tag="cdeg")
            nc.scalar.dma_start(out=cdeg_sb[:], in_=cdeg_d[:, :])

            colb_sb = crest_sb[:, o["colb"]:o["rec"]].bitcast(bf16)
            rec_sb = crest_sb[:, o["rec"]:o["rec"] + WPC]
            wvtb_sb = crest_sb[:, o["wvtb"]:o["wvtb"] + DOUT // 2].bitcast(bf16)
            iotab_sb = crest_sb[:, o["iotab"]:
                                o["iotab"] + P // 2].bitcast(bf16)
            degb_sb = cdeg_sb[0:1, o["degb"]:
                              o["degb"] + WPC * P // 2].bitcast(bf16)
            bvb_sb = cdeg_sb[0:1, o["bvb"]:o["bvb"] + DOUT // 2].bitcast(bf16)

            def epilogue(wl_abs, aggT_sb, q):
                out_ps = popool.tile([P, DOUT], f32, tag="outp")
                nc.tensor.matmul(out=out_ps[:],
                                 lhsT=aggT_sb[:, q * P:(q + 1) * P],
                                 rhs=wvtb_sb[:], start=True, stop=False)
                nc.tensor.matmul(out=out_ps[:],
                                 lhsT=degb_sb[0:1, wl_abs * P:(wl_abs + 1) * P],
                                 rhs=bvb_sb[0:1, :], start=False, stop=True)
                out_sb = ospool.tile([P, DOUT], f32, tag="outs")
                nc.scalar.mul(out=out_sb[:], in_=out_ps[:],
                              mul=rec_sb[:, wl_abs:wl_abs + 1])
                nc.scalar.dma_start(
                    out=out_d[wl_abs * P:(wl_abs + 1) * P, :], in_=out_sb[:])

            pending = []          # deferred (wl_abs, aggT_sb) epilogues
            for gi, g in enumerate(lay.groups):
                C = g["nchunks"]
                if gi == 0:
                    idx_sb = idx0_sb
                else:
                    f0, f1 = lay.idx_f32_off[gi], lay.idx_f32_off[gi + 1]
                    idx_sb = ipool.tile([P, f1 - f0], f32, tag="idx")
                    nc.sync.dma_start(out=idx_sb[:], in_=cidx_d[:, f0:f1])
                idx16_sb = idx_sb[:].bitcast(i16)

                xg = xgpool.tile([P, C * P], bf16, tag="xg")
                xg3 = xg[:].rearrange("p (c e) -> p c e", e=P)
                goff = 0
                for k, (h, cb, nchk) in enumerate(g["gathers"]):
                    ni = nchk * P
                    nc.gpsimd.dma_gather(
                        out_ap=xg3[:, cb:cb + nchk, :],
                        in_ap=(xlo_d if h == 0 else xhi_d)[:, :],
                        idxs_ap=idx16_sb[:, goff:goff + ni // 16],
                        num_idxs=ni,
                        num_idxs_reg=ni,
                        elem_size=DIN,
                        single_packet=bool(int(
                            os.environ.get("GAT_SP", "0"))),
                        queue_num=QCYC[k % 4],
                    )
                    goff += ni // 16

                # flush previous group's epilogues (inputs long ready)
                for args in pending:
                    epilogue(*args)
                pending = []

                # quads: 4 windows accumulate into slices of ONE psum bank
                # (one bank transition per quad instead of per window)
                wins = g["windows"]
                quads = [wins[i:i + 4] for i in range(0, len(wins), 4)]
                wq = 0
                for quad in quads:
                    Q = len(quad)
                    agg_ps = pspool.tile([P, Q * P], f32, tag="agg")
                    aggT_sb = atpool.tile([P, Q * P], bf16, tag="aggT")
                    for q, (chunklist, coff) in enumerate(quad):
                        wl_abs = int(g0s[gi]) + wq
                        wq += 1
                        T_w = len(chunklist)
                        oh = ohpool.tile([P, T_w * P], bf16, tag="oh")
                        nc.vector.tensor_tensor(
                            out=oh[:].rearrange("p (t j) -> p t j", j=P),
                            in0=iotab_sb[:, :P].rearrange(
                                "p (o j) -> p o j", j=P).to_broadcast(
                                [P, T_w, P]),
                            in1=colb_sb[:, coff:coff + T_w].to_broadcast(
                                [P, T_w, P]),
                            op=mybir.AluOpType.is_equal,
                        )
                        for t, xc in enumerate(chunklist):
                            nc.tensor.matmul(
                                out=agg_ps[:, q * P:(q + 1) * P],
                                lhsT=xg[:, xc * P:(xc + 1) * P],
                                rhs=oh[:, t * P:(t + 1) * P],
                                start=(t == 0),
                                stop=(t == T_w - 1),
                            )
                        pending.append((wl_abs, aggT_sb, q))
                    nc.scalar.copy(out=aggT_sb[:], in_=agg_ps[:])
            for args in pending:
                epilogue(*args)
    nc.compile()
    # Rewrite each gather's SWDGE queue as a pure function of its ASSIGNED
    # DMASW sem lane, so every lane is incremented by exactly one queue
    # (the ucode tracks sem ownership per queue).
    lane_q = (1, 2, 3, 0)
    for bb in nc.m.functions[0].blocks:
        for inst in bb.instructions:
            if 'DMAGatherAnt' not in type(inst).__name__:
                continue
            lane = None
            si = inst.sync_info
            if si is not None:
                for u in si.on_update:
                    n = u.ant_name
                    if n and n.startswith('DMASW'):
                        lane = int(n[5:].split('_')[0])
            assert lane is not None, "gather without DMASW sem"
            inst.queue_num = lane_q[lane % 4]
    return nc


def _put_bf16(arr, col_off, data_bf16):
    """Pack a bf16 [rows, n] block into f32 columns of arr at col_off."""
    rows, n = data_bf16.shape
    if n % 2:
        data_bf16 = np.concatenate(
            [data_bf16, np.zeros((rows, 1), data_bf16.dtype)], axis=1)
        n += 1
    tmp = np.zeros((rows, n // 2), np.float32)
    tmp.view(np.uint16).reshape(rows, n)[:] = data_bf16.view(np.uint16)
    arr[:rows, col_off:col_off + n // 2] = tmp


def _pack_const(lay, idx16, col_map, rec_map, deg_map, wvtb, bvb):
    """Returns (cidx, crest, cdeg) arrays for the constant tensors."""
    from ml_dtypes import bfloat16
    o = _offsets(lay)
    assert idx16.shape == (P, lay.ncid * 2), idx16.shape
    cidx = np.ascontiguousarray(idx16).view(np.float32)
    arr = np.zeros((P, o["CW"]), np.float32)
    _put_bf16(arr, o["colb"], col_map.astype(bfloat16))
    arr[:, o["rec"]:o["rec"] + WPC] = rec_map
    _put_bf16(arr, o["wvtb"], wvtb)
    iotab = np.broadcast_to(
        np.arange(P, dtype=np.float32)[None, :], (P, P)).astype(bfloat16)
    _put_bf16(arr, o["iotab"], np.ascontiguousarray(iotab))
    cdg = np.zeros((1, o["CD"]), np.float32)
    _put_bf16(cdg, o["degb"], deg_map.astype(bfloat16))
    _put_bf16(cdg, o["bvb"], bvb)
    return cidx, arr, cdg


def kernel(**inputs):
    global _last_exec_ns
    _ensure_ntff_hook()
    from concourse.bass_utils import run_bass_kernel_spmd
    from ml_dtypes import bfloat16

    x = np.ascontiguousarray(np.asarray(inputs["x"], dtype=np.float32))
    ei = np.asarray(inputs["edge_index"])
    row = np.asarray(ei[0]).astype(np.int64)
    col = np.asarray(ei[1]).astype(np.int64)
    Wv = np.asarray(inputs["Wv"], dtype=np.float32)
    bv = np.asarray(inputs["bv"], dtype=np.float32)

    xb = x.astype(bfloat16)
    wvtb = np.ascontiguousarray(Wv.T).astype(bfloat16)     # [DIN, DOUT]
    bvb = bv.reshape(1, DOUT).astype(bfloat16)

    lay, per_core = _prep(row, col)

    key = lay.key()
    if key not in _cache:
        _cache[key] = _build(lay)
    nc = _cache[key]

    xlo = np.ascontiguousarray(xb[:XLO])
    xhi = np.ascontiguousarray(xb[XLO:])
    in_maps = []
    for c in range(NCORES):
        cidx, crest, cdg = _pack_const(lay, *per_core[c], wvtb, bvb)
        in_maps.append({"xlo": xlo, "xhi": xhi, "cidx": cidx,
                        "crest": crest, "cdeg": cdg})

    trace = bool(os.environ.get("GAT_TRACE"))
    res = run_bass_kernel_spmd(nc, in_maps, list(range(NCORES)), trace=trace)
    _last_exec_ns = res.exec_time_ns
    globals()["_last_res"] = res

    out = np.concatenate([res.results[c]["out"] for c in range(NCORES)], axis=0)
    return np.ascontiguousarray(out[:N])
